# revision 81
# baseline (speedup 1.0000x reference)
"""Trainium2 Bass kernel for nn_Kernel_Conv (conv3x3+GELU -> per-pixel 19x19
conv -> conv3x3+sigmoid), SPMD over 8 NeuronCores.

Sharding: 8 cores = 2 batches x 4 H-slices (32 output rows each). All inputs
are host-preprocessed into per-core slabs (fp16) so the device program is
identical on every core.

Per-pixel conv: banded (Toeplitz) matmuls over w. For output row y and tap
row kh: out[c,w] += sum_w' xT[w',c] * M[w',w], with M[w',w] = ker[kh*19 +
(p-j+17), y, j-8] on the 19-diagonal band. Band tiles live in an SBUF arena
laid out (row, j_hi, kh, j_lo=2): j-pairs are interleaved so the matmul rhs
reads 4B-aligned element pairs at full SBUF fetch rate, while each
per-partition DMA run still covers a whole (j-window x 19 taps) block
(~1.9KB descriptors). Partition groups of G=32 share a 52-col window so one
dma_start serves 32 partitions. Arena zeros are written once per buffer via
f32-bitcast memsets on vector/gpsimd. conv1/conv2 are 4-way PE-column-tiled;
the x -> xT transpose uses PE transposes in just-in-time 8-row batches
(DMA xbar-transpose serializes against the band DMA stream, so PE wins).
"""

import sys
import types

for _p in ("/opt/trn_rl_repo",):
    if _p not in sys.path:
        sys.path.insert(0, _p)

import numpy as np
from contextlib import ExitStack

# Register the NTFF profile hook shim (harmless if tracing is never used)
try:
    import antenv  # noqa: F401
    if "antenv.axon_hooks" not in sys.modules:
        if "/root/.axon_site" not in sys.path:
            sys.path.insert(0, "/root/.axon_site")
        from trn_agent_boot.trn_boot import _ntff_profile_via_ctypes
        _hook = _ntff_profile_via_ctypes("/opt/axon/libaxon_pjrt.so")
        _mod = types.ModuleType("antenv.axon_hooks")
        _mod.get_axon_ntff_profile_hook = lambda: _hook
        sys.modules["antenv.axon_hooks"] = _mod
        antenv.axon_hooks = _mod
except Exception:
    pass

import concourse.bass as bass
import concourse.tile as tile
from concourse import bacc, mybir
from concourse.bass_utils import run_bass_kernel_spmd

F16 = np.float16

# ---------------- problem constants (hardcoded per the harness contract) ----
B, C, H, W = 2, 16, 128, 128
KK = 19            # per-pixel kernel size
NCORES = 8
HS = 32            # output rows per core
NY = 36            # y rows per core: [h0-2, h0+34)
NX = 56            # x (conv1 out) rows per core: [h0-11, h0+45)
NIN = 58           # input rows per core: [h0-12, h0+46)
G = 32             # band DMA partition-group size
JH = 26            # host window width in j-pairs (j window: [32g-2, 32g+50))
NG = 128 // G      # 4 groups
AW = 146           # arena j width; arena free layout is (r, j_hi, kh, j_lo)
NBLK = NY // 4     # 9 per-pixel chunks of 4 y rows
ROW = (AW // 2) * KK * 2   # arena elements per partition per y row
NBUF = 4           # band arena ring buffers (one 4-row chunk each)


def _host_prepare(input, kernel, w1, b1, w2, b2):
    """Build the per-core input slabs (numpy, fp16 except biases)."""
    inp = np.asarray(input, np.float32)
    ker = np.asarray(kernel, np.float32)

    # input, zero-padded: rows [-12, 140), cols [-1, 129)
    inp_pad = np.zeros((B, C, H + 26, W + 2), np.float32)
    inp_pad[:, :, 12:12 + H, 1:1 + W] = inp

    # conv weights as lhsT[(dx,c), o] per dy: [48, 3, 16]
    def wstack(wmat, order=(0, 1, 2)):
        ws = np.zeros((3, 48, 16), np.float32)
        for dy in range(3):
            for gi, dx in enumerate(order):
                ws[dy, gi * 16:gi * 16 + 16, :] = wmat[:, :, dy, dx].T
        return np.ascontiguousarray(ws.transpose(1, 0, 2)).astype(F16)

    w1s = wstack(np.asarray(w1, np.float32))
    w2s = wstack(np.asarray(w2, np.float32), order=(1, 0, 2))
    b1t = np.asarray(b1, np.float32).reshape(16, 1).copy()
    b2t = np.asarray(b2, np.float32).reshape(16, 1).copy()
    ident = np.eye(16, dtype=F16)

    # band data bandG[p, blk, r, jh, kh, jl] (j-pair-interleaved so the
    # matmul rhs reads 4B-aligned element pairs at full fetch rate):
    #   j = 32*(p//32) - 2 + 2*jh + jl; w = j - 8; kw = p - j + 17
    #   yr = h0-2+4blk+r; xr = yr+kh-9; value = ker[b, kh*19+kw, yr, w]
    p_i = np.arange(128)[:, None, None, None, None, None]
    blk_i = np.arange(NBLK)[None, :, None, None, None, None]
    r_i = np.arange(4)[None, None, :, None, None, None]
    jh_i = np.arange(JH)[None, None, None, :, None, None]
    kh_i = np.arange(KK)[None, None, None, None, :, None]
    jl_i = np.arange(2)[None, None, None, None, None, :]
    j_i = (p_i // G) * G - 2 + 2 * jh_i + jl_i
    kw_i = p_i - j_i + 17
    w_i = j_i - 8
    yg_i = 4 * blk_i + r_i
    base_valid = (kw_i >= 0) & (kw_i < KK) & (w_i >= 0) & (w_i < W)
    kw_c = np.clip(kw_i, 0, KK - 1)
    w_c = np.clip(w_i, 0, W - 1)
    pk_i = kh_i * KK + kw_c

    in_maps = []
    for cid in range(NCORES):
        b = cid // 4
        h0 = 32 * (cid % 4)

        # dx-tripled input slab [48, NIN, 128]
        inp3 = np.zeros((48, NIN, W), np.float32)
        rows = inp_pad[b, :, h0:h0 + NIN, :]  # global rows [h0-12, h0+46)
        for dx in range(3):
            inp3[dx * 16:dx * 16 + 16, :, :] = rows[:, :, dx:dx + W]
        inp3 = inp3.astype(F16)

        yr_i = h0 - 2 + yg_i
        xr_i = yr_i + kh_i - 9
        valid = base_valid & (yr_i >= 0) & (yr_i < H) & (xr_i >= 0) & (xr_i < H)
        yr_c = np.clip(yr_i, 0, H - 1)
        bandG = (ker[b][pk_i, yr_c, w_c] * valid).astype(F16)

        in_maps.append({
            "inp3": np.ascontiguousarray(inp3.reshape(48, NIN * W)),
            "bandG": np.ascontiguousarray(
                bandG.reshape(128, NBLK * 4 * JH * KK * 2)),
            "w1s": np.ascontiguousarray(w1s.reshape(48, 3 * 16)),
            "w2s": np.ascontiguousarray(w2s.reshape(48, 3 * 16)),
            "b1t": b1t,
            "b2t": b2t,
            "ident": ident,
        })
    return in_maps


def _build_program():
    nc = bacc.Bacc("TRN2", target_bir_lowering=False, debug=False,
                   num_devices=NCORES)
    dt = mybir.dt

    inp3_d = nc.dram_tensor("inp3", [48, NIN * W], dt.float16,
                            kind="ExternalInput").ap()
    bandG_d = nc.dram_tensor("bandG", [128, NBLK * 4 * JH * KK * 2],
                             dt.float16, kind="ExternalInput").ap()
    w1s_d = nc.dram_tensor("w1s", [48, 3 * 16], dt.float16,
                           kind="ExternalInput").ap()
    w2s_d = nc.dram_tensor("w2s", [48, 3 * 16], dt.float16,
                           kind="ExternalInput").ap()
    b1t_d = nc.dram_tensor("b1t", [16, 1], dt.float32, kind="ExternalInput").ap()
    b2t_d = nc.dram_tensor("b2t", [16, 1], dt.float32, kind="ExternalInput").ap()
    ident_d = nc.dram_tensor("ident", [16, 16], dt.float16,
                             kind="ExternalInput").ap()
    out_d = nc.dram_tensor("out", [16, HS * W], dt.float16,
                           kind="ExternalOutput").ap()

    with tile.TileContext(nc) as tc:
        with ExitStack() as ctx:
            _body(ctx, tc, inp3_d, bandG_d, w1s_d, w2s_d, b1t_d, b2t_d,
                  ident_d, out_d)
    nc.compile()
    return nc


def _body(ctx, tc, inp3_d, bandG_d, w1s_d, w2s_d, b1t_d, b2t_d, ident_d,
          out_d):
    nc = tc.nc
    dt = mybir.dt
    AFT = mybir.ActivationFunctionType

    consts = ctx.enter_context(tc.tile_pool(name="consts", bufs=1))
    bigs = ctx.enter_context(tc.tile_pool(name="bigs", bufs=1))
    ps_c1 = ctx.enter_context(tc.tile_pool(name="ps_c1", bufs=2, space="PSUM"))
    ps_tp = ctx.enter_context(tc.tile_pool(name="ps_tp", bufs=1, space="PSUM"))
    ps_pp = ctx.enter_context(tc.tile_pool(name="ps_pp", bufs=3, space="PSUM"))
    ps_c2 = ctx.enter_context(tc.tile_pool(name="ps_c2", bufs=2, space="PSUM"))

    # ---- persistent SBUF tiles -------------------------------------------
    w1s_t = consts.tile([48, 3 * 16], dt.float16, tag="w1s")
    w2s_t = consts.tile([48, 3 * 16], dt.float16, tag="w2s")
    b1_t = consts.tile([16, 1], dt.float32, tag="b1")
    b2_t = consts.tile([16, 1], dt.float32, tag="b2")
    id_t = consts.tile([16, 16], dt.float16, tag="ident")
    inp3_t = bigs.tile([48, NIN * W], dt.float16, tag="inp3")
    x_t = bigs.tile([16, NX * W], dt.float16, tag="x")
    xT_t = bigs.tile([128, NX * 16], dt.float16, tag="xT")
    y3_t = bigs.tile([48, NY * W], dt.float16, tag="y3")
    out_t = bigs.tile([16, HS * W], dt.float16, tag="out")
    band = [bigs.tile([128, 4 * ROW], dt.float16, tag=f"band{i}",
                      name=f"band{i}") for i in range(NBUF)]
    band_v = [t[:].rearrange("p (r jh kh jl) -> p r jh kh jl",
                             r=4, jh=AW // 2, kh=KK, jl=2) for t in band]


    # ---- input loads (sync queue; inp3 split so conv1 rounds never wait) -
    C1, C2 = 20 * W, 36 * W
    nc.sync.dma_start(inp3_t[:, 0:C1], inp3_d[:, 0:C1])
    nc.sync.dma_start(w1s_t[:], w1s_d)
    nc.sync.dma_start(b1_t[:], b1t_d)
    nc.sync.dma_start(inp3_t[:, C1:C2], inp3_d[:, C1:C2])
    nc.sync.dma_start(inp3_t[:, C2:], inp3_d[:, C2:])
    nc.sync.dma_start(w2s_t[:], w2s_d)
    nc.sync.dma_start(b2_t[:], b2t_d)
    nc.sync.dma_start(id_t[:], ident_d)

    # ---- one-time zeroing (wide bitcast views cut memset element count;
    # buffer 0 on vector, the rest on the otherwise-idle gpsimd queue so
    # the xT copies aren't stuck behind memsets in the vector FIFO) --------
    for i in range(NBUF):
        eng = nc.vector if i % 2 == 0 else nc.gpsimd
        eng.memset(band[i][:].bitcast(dt.float32), 0.0)
    y3_v = y3_t[:].rearrange("p (r w) -> p r w", r=NY)
    nc.vector.memset(y3_v[0:48, :, 0:1], 0.0)
    nc.vector.memset(y3_v[0:48, :, W - 1:W], 0.0)

    bandG_v = bandG_d.rearrange("p (blk r jh kh jl) -> p blk r jh kh jl",
                                blk=NBLK, r=4, jh=JH, kh=KK, jl=2)

    def band_chunk_dma(blk):
        # group g window: j in [32g-2, 32g+50) -> j_hi in [16g-1, 16g+25);
        # g=0 clips the two j<0 columns. y rows 0 (chunk 0) and 35 (chunk 8)
        # are never read by conv2, so skip their transfers (the pp matmuls
        # for them then see memset zeros / stale data, discarded anyway).
        dstt = band_v[blk % NBUF]
        r0, r1 = (1, 4) if blk == 0 else (0, 3) if blk == NBLK - 1 else (0, 4)
        # late chunks: split issue across both HWDGE rings (scalar's conv1
        # activation chain is long done by the time these are issued)
        engs = [nc.sync, nc.sync, nc.scalar, nc.scalar] if blk >= 5 \
            else [nc.sync] * NG
        for g in range(NG):
            jh0 = max(0, 16 * g - 1)
            s0 = 1 if g == 0 else 0
            engs[g].dma_start(
                dstt[G * g: G * g + G, r0:r1, jh0: 16 * g + 25, :, :],
                bandG_v[G * g: G * g + G, blk, r0:r1, s0:JH, :, :])

    for pre in range(NBUF):
        band_chunk_dma(pre)

    # ---- conv1 + GELU -> x, 4-way column-tiled ---------------------------
    inp3_v = inp3_t[:].rearrange("p (r w) -> p r w", r=NIN)
    x_v = x_t[:].rearrange("p (r w) -> p r w", r=NX)

    def transpose_batch(tb, nrows=8):
        # x rows [8tb, 8tb+nrows) -> xT cols, via PE transposes + DVE copy
        pt = ps_tp.tile([128, 128], dt.float16, tag="tp")
        for rr in range(nrows):
            nc.tensor.transpose(pt[:, 16 * rr: 16 * rr + 16],
                                x_v[:, 8 * tb + rr, :], id_t[:])
        nc.vector.tensor_copy(
            xT_t[:, 128 * tb: 128 * tb + 16 * nrows], pt[:, 0:16 * nrows])

    for sr in range(4):
        nsub = 4 if sr < 3 else 2
        psum = ps_c1.tile([128, 512], dt.float32, tag="c1")
        for sub in range(nsub):
            b4 = 4 * sr + sub
            for dy in range(3):
                nc.tensor.matmul(
                    psum[32 * sub: 32 * sub + 16, :],
                    w1s_t[:, dy * 16:(dy + 1) * 16],
                    inp3_v[:, 4 * b4 + dy: 4 * b4 + dy + 4, :],
                    start=(dy == 0), stop=(dy == 2),
                    tile_position=(0, 32 * sub))
        for sub in range(nsub):
            b4 = 4 * sr + sub
            nc.scalar.activation(x_t[:, 512 * b4: 512 * (b4 + 1)],
                                 psum[32 * sub: 32 * sub + 16, :],
                                 AFT.Gelu, bias=b1_t[:])
        if sr == 0:
            transpose_batch(0)     # x rows 0-7 (ACT blocks 0-1 done)
        if sr == 1:
            transpose_batch(1)     # rows 8-23 (ACT blocks 2-5)
            transpose_batch(2)
        if sr == 2:
            transpose_batch(3)     # rows 24-31 (ACT blocks 6-7)

    # ---- per-pixel conv + interleaved conv2 ------------------------------
    def conv2_round(blocks):
        psum = ps_c2.tile([128, 512], dt.float32, tag="c2")
        for sub, b in enumerate(blocks):
            for dy in range(3):
                nc.tensor.matmul(
                    psum[32 * sub: 32 * sub + 16, :],
                    w2s_t[:, dy * 16:(dy + 1) * 16],
                    y3_v[:, 4 * b + 1 + dy: 4 * b + 5 + dy, :],
                    start=(dy == 0), stop=(dy == 2),
                    tile_position=(0, 32 * sub))
        for sub, b in enumerate(blocks):
            nc.scalar.activation(out_t[:, 512 * b: 512 * (b + 1)],
                                 psum[32 * sub: 32 * sub + 16, :],
                                 AFT.Sigmoid, bias=b2_t[:])

    def shift_wave(r0, r1):
        # dx-shifted y copies for conv2 (edge cols stay zero from memset)
        nc.sync.dma_start(y3_v[16:32, r0:r1, 1:W], y3_v[0:16, r0:r1, 0:W - 1])
        nc.sync.dma_start(y3_v[32:48, r0:r1, 0:W - 1], y3_v[0:16, r0:r1, 1:W])

    for blk in range(NBLK):
        bt = band_v[blk % NBUF]
        pp = ps_pp.tile([128, 128], dt.float32, tag="pp")
        # y rows 0 (blk 0, g 0) and 35 (blk 8, g 3) are never read by conv2
        g_lo = 1 if blk == 0 else 0
        g_hi = 3 if blk == NBLK - 1 else 4
        for t in range(22):
            xg = 4 * blk + t
            lhs = xT_t[:, 16 * xg: 16 * xg + 16]
            for g in range(max(g_lo, t - 18), min(g_hi, t + 1)):
                kh = t - g
                nc.tensor.matmul(
                    pp[32 * g: 32 * g + 16, :],
                    lhs,
                    bt[:, g, 4:4 + W // 2, kh, :],
                    start=(kh == 0), stop=(kh == KK - 1),
                    tile_position=(0, 32 * g))
        # evict y rows (f32 psum -> fp16 y3)
        for g in range(g_lo, g_hi):
            yg = 4 * blk + g
            nc.vector.tensor_copy(y3_v[0:16, yg, :], pp[32 * g: 32 * g + 16, :])
        if blk + NBUF < NBLK:
            band_chunk_dma(blk + NBUF)
        # JIT transpose batches: blk3 needs x rows <=33 (tb4), blk5 <=41
        # (tb5), blk7 <=49 + blk8 <=53 (tb6)
        if blk == 0:
            transpose_batch(4)
        if blk == 2:
            transpose_batch(5)
        if blk == 4:
            transpose_batch(6, nrows=6)  # x rows 54,55 are never read
        if blk == 6:
            shift_wave(1, 28)
            conv2_round([0, 1, 2, 3])
        if blk == 7:
            conv2_round([4, 5])
            nc.scalar.dma_start(out_d[:, 0:512 * 4], out_t[:, 0:512 * 4])

    shift_wave(28, NY - 1)
    conv2_round([6, 7])
    nc.scalar.dma_start(out_d[:, 512 * 4:], out_t[:, 512 * 4:])


_NC_CACHE = None
LAST = {}


def _get_nc():
    global _NC_CACHE
    if _NC_CACHE is None:
        _NC_CACHE = _build_program()
    return _NC_CACHE


def kernel(input, kernel, w1, b1, w2, b2, _trace=False, _tmpdir=None):
    in_maps = _host_prepare(input, kernel, w1, b1, w2, b2)
    nc = _get_nc()
    res = run_bass_kernel_spmd(nc, in_maps, core_ids=list(range(NCORES)),
                               trace=_trace, tmpdir=_tmpdir)
    out = np.zeros((B, C, H, W), np.float32)
    for cid in range(NCORES):
        b = cid // 4
        h0 = 32 * (cid % 4)
        out[b, :, h0:h0 + HS, :] = (
            res.results[cid]["out"].astype(np.float32).reshape(16, HS, W))
    LAST["exec_ns"] = res.exec_time_ns
    LAST["trace"] = res.instructions_and_trace
    return out


# revision 83
# speedup vs baseline: 1.0033x; 1.0033x over previous
"""Trainium2 Bass kernel for nn_Kernel_Conv (conv3x3+GELU -> per-pixel 19x19
conv -> conv3x3+sigmoid), SPMD over 8 NeuronCores.

Sharding: 8 cores = 2 batches x 4 H-slices (32 output rows each). All inputs
are host-preprocessed into per-core slabs (fp16) so the device program is
identical on every core.

Per-pixel conv: banded (Toeplitz) matmuls over w. For output row y and tap
row kh: out[c,w] += sum_w' xT[w',c] * M[w',w], with M[w',w] = ker[kh*19 +
(p-j+17), y, j-8] on the 19-diagonal band. Band tiles live in an SBUF arena
laid out (row, j_hi, kh, j_lo=2): j-pairs are interleaved so the matmul rhs
reads 4B-aligned element pairs at full SBUF fetch rate, while each
per-partition DMA run still covers a whole (j-window x 19 taps) block
(~1.9KB descriptors). Partition groups of G=32 share a 52-col window so one
dma_start serves 32 partitions. Arena zeros are written once per buffer via
f32-bitcast memsets on vector/gpsimd. conv1/conv2 are 4-way PE-column-tiled;
the x -> xT transpose uses PE transposes in just-in-time 8-row batches
(DMA xbar-transpose serializes against the band DMA stream, so PE wins).
"""

import sys
import types

for _p in ("/opt/trn_rl_repo",):
    if _p not in sys.path:
        sys.path.insert(0, _p)

import numpy as np
from contextlib import ExitStack

# Register the NTFF profile hook shim (harmless if tracing is never used)
try:
    import antenv  # noqa: F401
    if "antenv.axon_hooks" not in sys.modules:
        if "/root/.axon_site" not in sys.path:
            sys.path.insert(0, "/root/.axon_site")
        from trn_agent_boot.trn_boot import _ntff_profile_via_ctypes
        _hook = _ntff_profile_via_ctypes("/opt/axon/libaxon_pjrt.so")
        _mod = types.ModuleType("antenv.axon_hooks")
        _mod.get_axon_ntff_profile_hook = lambda: _hook
        sys.modules["antenv.axon_hooks"] = _mod
        antenv.axon_hooks = _mod
except Exception:
    pass

import concourse.bass as bass
import concourse.tile as tile
from concourse import bacc, mybir
from concourse.bass_utils import run_bass_kernel_spmd

F16 = np.float16

# ---------------- problem constants (hardcoded per the harness contract) ----
B, C, H, W = 2, 16, 128, 128
KK = 19            # per-pixel kernel size
NCORES = 8
HS = 32            # output rows per core
NY = 36            # y rows per core: [h0-2, h0+34)
NX = 56            # x (conv1 out) rows per core: [h0-11, h0+45)
NIN = 58           # input rows per core: [h0-12, h0+46)
G = 32             # band DMA partition-group size
JH = 26            # host window width in j-pairs (j window: [32g-2, 32g+50))
NG = 128 // G      # 4 groups
AW = 146           # arena j width; arena free layout is (r, j_hi, kh, j_lo)
NBLK = NY // 4     # 9 per-pixel chunks of 4 y rows
ROW = (AW // 2) * KK * 2   # arena elements per partition per y row
NBUF = 4           # band arena ring buffers (one 4-row chunk each)


def _host_prepare(input, kernel, w1, b1, w2, b2):
    """Build the per-core input slabs (numpy, fp16 except biases)."""
    inp = np.asarray(input, np.float32)
    ker = np.asarray(kernel, np.float32)

    # input, zero-padded: rows [-12, 140), cols [-1, 129)
    inp_pad = np.zeros((B, C, H + 26, W + 2), np.float32)
    inp_pad[:, :, 12:12 + H, 1:1 + W] = inp

    # conv weights as lhsT[(dx,c), o] per dy: [48, 3, 16]
    def wstack(wmat, order=(0, 1, 2)):
        ws = np.zeros((3, 48, 16), np.float32)
        for dy in range(3):
            for gi, dx in enumerate(order):
                ws[dy, gi * 16:gi * 16 + 16, :] = wmat[:, :, dy, dx].T
        return np.ascontiguousarray(ws.transpose(1, 0, 2)).astype(F16)

    w1s = wstack(np.asarray(w1, np.float32))
    w2s = wstack(np.asarray(w2, np.float32), order=(1, 0, 2))
    b1t = np.asarray(b1, np.float32).reshape(16, 1).copy()
    b2t = np.asarray(b2, np.float32).reshape(16, 1).copy()
    ident = np.eye(16, dtype=F16)

    # band data bandG[p, blk, r, jh, kh, jl] (j-pair-interleaved so the
    # matmul rhs reads 4B-aligned element pairs at full fetch rate):
    #   j = 32*(p//32) - 2 + 2*jh + jl; w = j - 8; kw = p - j + 17
    #   yr = h0-2+4blk+r; xr = yr+kh-9; value = ker[b, kh*19+kw, yr, w]
    p_i = np.arange(128)[:, None, None, None, None, None]
    blk_i = np.arange(NBLK)[None, :, None, None, None, None]
    r_i = np.arange(4)[None, None, :, None, None, None]
    jh_i = np.arange(JH)[None, None, None, :, None, None]
    kh_i = np.arange(KK)[None, None, None, None, :, None]
    jl_i = np.arange(2)[None, None, None, None, None, :]
    j_i = (p_i // G) * G - 2 + 2 * jh_i + jl_i
    kw_i = p_i - j_i + 17
    w_i = j_i - 8
    yg_i = 4 * blk_i + r_i
    base_valid = (kw_i >= 0) & (kw_i < KK) & (w_i >= 0) & (w_i < W)
    kw_c = np.clip(kw_i, 0, KK - 1)
    w_c = np.clip(w_i, 0, W - 1)
    pk_i = kh_i * KK + kw_c

    in_maps = []
    for cid in range(NCORES):
        b = cid // 4
        h0 = 32 * (cid % 4)

        # dx-tripled input slab [48, NIN, 128]
        inp3 = np.zeros((48, NIN, W), np.float32)
        rows = inp_pad[b, :, h0:h0 + NIN, :]  # global rows [h0-12, h0+46)
        for dx in range(3):
            inp3[dx * 16:dx * 16 + 16, :, :] = rows[:, :, dx:dx + W]
        inp3 = inp3.astype(F16)

        yr_i = h0 - 2 + yg_i
        xr_i = yr_i + kh_i - 9
        valid = base_valid & (yr_i >= 0) & (yr_i < H) & (xr_i >= 0) & (xr_i < H)
        yr_c = np.clip(yr_i, 0, H - 1)
        bandG = (ker[b][pk_i, yr_c, w_c] * valid).astype(F16)

        in_maps.append({
            "inp3": np.ascontiguousarray(inp3.reshape(48, NIN * W)),
            "bandG": np.ascontiguousarray(
                bandG.reshape(128, NBLK * 4 * JH * KK * 2)),
            "w1s": np.ascontiguousarray(w1s.reshape(48, 3 * 16)),
            "w2s": np.ascontiguousarray(w2s.reshape(48, 3 * 16)),
            "b1t": b1t,
            "b2t": b2t,
            "ident": ident,
        })
    return in_maps


def _build_program():
    nc = bacc.Bacc("TRN2", target_bir_lowering=False, debug=False,
                   num_devices=NCORES)
    dt = mybir.dt

    inp3_d = nc.dram_tensor("inp3", [48, NIN * W], dt.float16,
                            kind="ExternalInput").ap()
    bandG_d = nc.dram_tensor("bandG", [128, NBLK * 4 * JH * KK * 2],
                             dt.float16, kind="ExternalInput").ap()
    w1s_d = nc.dram_tensor("w1s", [48, 3 * 16], dt.float16,
                           kind="ExternalInput").ap()
    w2s_d = nc.dram_tensor("w2s", [48, 3 * 16], dt.float16,
                           kind="ExternalInput").ap()
    b1t_d = nc.dram_tensor("b1t", [16, 1], dt.float32, kind="ExternalInput").ap()
    b2t_d = nc.dram_tensor("b2t", [16, 1], dt.float32, kind="ExternalInput").ap()
    ident_d = nc.dram_tensor("ident", [16, 16], dt.float16,
                             kind="ExternalInput").ap()
    out_d = nc.dram_tensor("out", [16, HS * W], dt.float16,
                           kind="ExternalOutput").ap()

    with tile.TileContext(nc) as tc:
        with ExitStack() as ctx:
            _body(ctx, tc, inp3_d, bandG_d, w1s_d, w2s_d, b1t_d, b2t_d,
                  ident_d, out_d)
    nc.compile()
    return nc


def _body(ctx, tc, inp3_d, bandG_d, w1s_d, w2s_d, b1t_d, b2t_d, ident_d,
          out_d):
    nc = tc.nc
    dt = mybir.dt
    AFT = mybir.ActivationFunctionType

    consts = ctx.enter_context(tc.tile_pool(name="consts", bufs=1))
    bigs = ctx.enter_context(tc.tile_pool(name="bigs", bufs=1))
    ps_c1 = ctx.enter_context(tc.tile_pool(name="ps_c1", bufs=3, space="PSUM"))
    ps_tp = ctx.enter_context(tc.tile_pool(name="ps_tp", bufs=1, space="PSUM"))
    ps_pp = ctx.enter_context(tc.tile_pool(name="ps_pp", bufs=2, space="PSUM"))
    ps_c2 = ctx.enter_context(tc.tile_pool(name="ps_c2", bufs=2, space="PSUM"))

    # ---- persistent SBUF tiles -------------------------------------------
    w1s_t = consts.tile([48, 3 * 16], dt.float16, tag="w1s")
    w2s_t = consts.tile([48, 3 * 16], dt.float16, tag="w2s")
    b1_t = consts.tile([16, 1], dt.float32, tag="b1")
    b2_t = consts.tile([16, 1], dt.float32, tag="b2")
    id_t = consts.tile([16, 16], dt.float16, tag="ident")
    inp3_t = bigs.tile([48, NIN * W], dt.float16, tag="inp3")
    x_t = bigs.tile([16, NX * W], dt.float16, tag="x")
    xT_t = bigs.tile([128, NX * 16], dt.float16, tag="xT")
    y3_t = bigs.tile([48, NY * W], dt.float16, tag="y3")
    out_t = bigs.tile([16, HS * W], dt.float16, tag="out")
    band = [bigs.tile([128, 4 * ROW], dt.float16, tag=f"band{i}",
                      name=f"band{i}") for i in range(NBUF)]
    band_v = [t[:].rearrange("p (r jh kh jl) -> p r jh kh jl",
                             r=4, jh=AW // 2, kh=KK, jl=2) for t in band]


    # ---- input loads (sync queue; inp3 split so conv1 rounds never wait) -
    C1, C2 = 20 * W, 36 * W
    nc.sync.dma_start(inp3_t[:, 0:C1], inp3_d[:, 0:C1])
    nc.sync.dma_start(w1s_t[:], w1s_d)
    nc.sync.dma_start(b1_t[:], b1t_d)
    nc.sync.dma_start(inp3_t[:, C1:C2], inp3_d[:, C1:C2])
    nc.sync.dma_start(inp3_t[:, C2:], inp3_d[:, C2:])
    nc.sync.dma_start(w2s_t[:], w2s_d)
    nc.sync.dma_start(b2_t[:], b2t_d)
    nc.sync.dma_start(id_t[:], ident_d)

    # ---- one-time zeroing (wide bitcast views cut memset element count;
    # buffer 0 on vector, the rest on the otherwise-idle gpsimd queue so
    # the xT copies aren't stuck behind memsets in the vector FIFO) --------
    # only the matmul-read region (j-pairs [4, 68)) ever needs zeros
    for i in range(NBUF):
        eng = nc.vector if i % 2 == 0 else nc.gpsimd
        zv = band_v[i][:, :, 4:4 + W // 2, :, :]
        eng.memset(zv.bitcast(dt.float32), 0.0)
    y3_v = y3_t[:].rearrange("p (r w) -> p r w", r=NY)
    nc.vector.memset(y3_v[0:48, :, 0:1], 0.0)
    nc.vector.memset(y3_v[0:48, :, W - 1:W], 0.0)

    bandG_v = bandG_d.rearrange("p (blk r jh kh jl) -> p blk r jh kh jl",
                                blk=NBLK, r=4, jh=JH, kh=KK, jl=2)

    def band_chunk_dma(blk):
        # group g window: j in [32g-2, 32g+50) -> j_hi in [16g-1, 16g+25);
        # g=0 clips the two j<0 columns. y rows 0 (chunk 0) and 35 (chunk 8)
        # are never read by conv2, so skip their transfers (the pp matmuls
        # for them then see memset zeros / stale data, discarded anyway).
        dstt = band_v[blk % NBUF]
        r0, r1 = (1, 4) if blk == 0 else (0, 3) if blk == NBLK - 1 else (0, 4)
        # late chunks: split issue across both HWDGE rings (scalar's conv1
        # activation chain is long done by the time these are issued)
        engs = [nc.sync, nc.sync, nc.scalar, nc.scalar] if blk >= 5 \
            else [nc.sync] * NG
        for g in range(NG):
            jh0 = max(0, 16 * g - 1)
            s0 = 1 if g == 0 else 0
            engs[g].dma_start(
                dstt[G * g: G * g + G, r0:r1, jh0: 16 * g + 25, :, :],
                bandG_v[G * g: G * g + G, blk, r0:r1, s0:JH, :, :])

    for pre in range(NBUF):
        band_chunk_dma(pre)

    # ---- conv1 + GELU -> x, 4-way column-tiled ---------------------------
    inp3_v = inp3_t[:].rearrange("p (r w) -> p r w", r=NIN)
    x_v = x_t[:].rearrange("p (r w) -> p r w", r=NX)

    def transpose_batch(tb, nrows=8):
        # x rows [8tb, 8tb+nrows) -> xT cols, via PE transposes + DVE copy
        pt = ps_tp.tile([128, 128], dt.float16, tag="tp")
        for rr in range(nrows):
            nc.tensor.transpose(pt[:, 16 * rr: 16 * rr + 16],
                                x_v[:, 8 * tb + rr, :], id_t[:])
        nc.vector.tensor_copy(
            xT_t[:, 128 * tb: 128 * tb + 16 * nrows], pt[:, 0:16 * nrows])

    for sr in range(4):
        nsub = 4 if sr < 3 else 2
        psum = ps_c1.tile([128, 512], dt.float32, tag="c1")
        for sub in range(nsub):
            b4 = 4 * sr + sub
            for dy in range(3):
                nc.tensor.matmul(
                    psum[32 * sub: 32 * sub + 16, :],
                    w1s_t[:, dy * 16:(dy + 1) * 16],
                    inp3_v[:, 4 * b4 + dy: 4 * b4 + dy + 4, :],
                    start=(dy == 0), stop=(dy == 2),
                    tile_position=(0, 32 * sub))
        for sub in range(nsub):
            b4 = 4 * sr + sub
            nc.scalar.activation(x_t[:, 512 * b4: 512 * (b4 + 1)],
                                 psum[32 * sub: 32 * sub + 16, :],
                                 AFT.Gelu, bias=b1_t[:])
        if sr == 0:
            transpose_batch(0)     # x rows 0-7 (ACT blocks 0-1 done)
        if sr == 1:
            transpose_batch(1)     # rows 8-23 (ACT blocks 2-5)
            transpose_batch(2)
        if sr == 2:
            transpose_batch(3)     # rows 24-31 (ACT blocks 6-7)

    # ---- per-pixel conv + interleaved conv2 ------------------------------
    def conv2_round(blocks):
        psum = ps_c2.tile([128, 512], dt.float32, tag="c2")
        for sub, b in enumerate(blocks):
            for dy in range(3):
                nc.tensor.matmul(
                    psum[32 * sub: 32 * sub + 16, :],
                    w2s_t[:, dy * 16:(dy + 1) * 16],
                    y3_v[:, 4 * b + 1 + dy: 4 * b + 5 + dy, :],
                    start=(dy == 0), stop=(dy == 2),
                    tile_position=(0, 32 * sub))
        for sub, b in enumerate(blocks):
            nc.scalar.activation(out_t[:, 512 * b: 512 * (b + 1)],
                                 psum[32 * sub: 32 * sub + 16, :],
                                 AFT.Sigmoid, bias=b2_t[:])

    def shift_wave(r0, r1):
        # dx-shifted y copies for conv2 (edge cols stay zero from memset)
        nc.sync.dma_start(y3_v[16:32, r0:r1, 1:W], y3_v[0:16, r0:r1, 0:W - 1])
        nc.sync.dma_start(y3_v[32:48, r0:r1, 0:W - 1], y3_v[0:16, r0:r1, 1:W])

    for blk in range(NBLK):
        bt = band_v[blk % NBUF]
        pp = ps_pp.tile([128, 128], dt.float32, tag="pp")
        for t in range(22):
            xg = 4 * blk + t
            lhs = xT_t[:, 16 * xg: 16 * xg + 16]
            for g in range(max(0, t - 18), min(3, t) + 1):
                kh = t - g
                nc.tensor.matmul(
                    pp[32 * g: 32 * g + 16, :],
                    lhs,
                    bt[:, g, 4:4 + W // 2, kh, :],
                    start=(kh == 0), stop=(kh == KK - 1),
                    tile_position=(0, 32 * g))
        # evict 4 y rows (f32 psum -> fp16 y3)
        for g in range(4):
            yg = 4 * blk + g
            nc.vector.tensor_copy(y3_v[0:16, yg, :], pp[32 * g: 32 * g + 16, :])
        if blk + NBUF < NBLK:
            band_chunk_dma(blk + NBUF)
        # JIT transpose batches: blk3 needs x rows <=33 (tb4), blk5 <=41
        # (tb5), blk7 <=49 + blk8 <=53 (tb6)
        if blk == 0:
            transpose_batch(4)
        if blk == 2:
            transpose_batch(5)
        if blk == 4:
            transpose_batch(6, nrows=6)  # x rows 54,55 are never read
        if blk == 6:
            shift_wave(0, 28)
            conv2_round([0, 1, 2, 3])
        if blk == 7:
            conv2_round([4, 5])
            nc.scalar.dma_start(out_d[:, 0:512 * 4], out_t[:, 0:512 * 4])

    shift_wave(28, NY)
    conv2_round([6, 7])
    nc.scalar.dma_start(out_d[:, 512 * 4:], out_t[:, 512 * 4:])


_NC_CACHE = None
LAST = {}


def _get_nc():
    global _NC_CACHE
    if _NC_CACHE is None:
        _NC_CACHE = _build_program()
    return _NC_CACHE


def kernel(input, kernel, w1, b1, w2, b2, _trace=False, _tmpdir=None):
    in_maps = _host_prepare(input, kernel, w1, b1, w2, b2)
    nc = _get_nc()
    res = run_bass_kernel_spmd(nc, in_maps, core_ids=list(range(NCORES)),
                               trace=_trace, tmpdir=_tmpdir)
    out = np.zeros((B, C, H, W), np.float32)
    for cid in range(NCORES):
        b = cid // 4
        h0 = 32 * (cid % 4)
        out[b, :, h0:h0 + HS, :] = (
            res.results[cid]["out"].astype(np.float32).reshape(16, HS, W))
    LAST["exec_ns"] = res.exec_time_ns
    LAST["trace"] = res.instructions_and_trace
    return out


# revision 85
# speedup vs baseline: 1.0348x; 1.0314x over previous
"""Trainium2 Bass kernel for nn_Kernel_Conv (conv3x3+GELU -> per-pixel 19x19
conv -> conv3x3+sigmoid), SPMD over 8 NeuronCores.

Sharding: 8 cores = 2 batches x 4 H-slices (32 output rows each). All inputs
are host-preprocessed into per-core slabs (fp16) so the device program is
identical on every core.

Per-pixel conv: banded (Toeplitz) matmuls over w. For output row y and tap
row kh: out[c,w] += sum_w' xT[w',c] * M[w',w], with M[w',w] = ker[kh*19 +
(p-j+17), y, j-8] on the 19-diagonal band. Band tiles live in an SBUF arena
laid out (row, j_hi, kh, j_lo=2): j-pairs are interleaved so the matmul rhs
reads 4B-aligned element pairs at full SBUF fetch rate, while each
per-partition DMA run still covers a whole (j-window x 19 taps) block
(~1.9KB descriptors). Partition groups of G=32 share a 52-col window so one
dma_start serves 32 partitions. Arena zeros are written once per buffer via
f32-bitcast memsets on vector/gpsimd. conv1/conv2 are 4-way PE-column-tiled;
the x -> xT transpose uses PE transposes in just-in-time 8-row batches
(DMA xbar-transpose serializes against the band DMA stream, so PE wins).
"""

import sys
import types

for _p in ("/opt/trn_rl_repo",):
    if _p not in sys.path:
        sys.path.insert(0, _p)

import numpy as np
from contextlib import ExitStack

# Register the NTFF profile hook shim (harmless if tracing is never used)
try:
    import antenv  # noqa: F401
    if "antenv.axon_hooks" not in sys.modules:
        if "/root/.axon_site" not in sys.path:
            sys.path.insert(0, "/root/.axon_site")
        from trn_agent_boot.trn_boot import _ntff_profile_via_ctypes
        _hook = _ntff_profile_via_ctypes("/opt/axon/libaxon_pjrt.so")
        _mod = types.ModuleType("antenv.axon_hooks")
        _mod.get_axon_ntff_profile_hook = lambda: _hook
        sys.modules["antenv.axon_hooks"] = _mod
        antenv.axon_hooks = _mod
except Exception:
    pass

import concourse.bass as bass
import concourse.tile as tile
from concourse import bacc, mybir
from concourse.bass_utils import run_bass_kernel_spmd

F16 = np.float16

# ---------------- problem constants (hardcoded per the harness contract) ----
B, C, H, W = 2, 16, 128, 128
KK = 19            # per-pixel kernel size
NCORES = 8
HS = 32            # output rows per core
NY = 36            # y rows per core: [h0-2, h0+34)
NX = 56            # x (conv1 out) rows per core: [h0-11, h0+45)
NIN = 58           # input rows per core: [h0-12, h0+46)
G = 32             # band DMA partition-group size
JH = 26            # host window width in j-pairs (j window: [32g-2, 32g+50))
NG = 128 // G      # 4 groups
AW = 146           # arena j width; arena free layout is (r, j_hi, kh, j_lo)
NBLK = NY // 4     # 9 per-pixel chunks of 4 y rows
ROW = (AW // 2) * KK * 2   # arena elements per partition per y row
NBUF = 4           # band arena ring buffers (one 4-row chunk each)


def _host_prepare(input, kernel, w1, b1, w2, b2):
    """Build the per-core input slabs (numpy, fp16 except biases)."""
    inp = np.asarray(input, np.float32)
    ker = np.asarray(kernel, np.float32)

    # input, zero-padded: rows [-12, 140), cols [-1, 129)
    inp_pad = np.zeros((B, C, H + 26, W + 2), np.float32)
    inp_pad[:, :, 12:12 + H, 1:1 + W] = inp

    # conv weights as lhsT[(dx,c), o] per dy: [48, 3, 16]
    def wstack(wmat, order=(0, 1, 2)):
        ws = np.zeros((3, 48, 16), np.float32)
        for dy in range(3):
            for gi, dx in enumerate(order):
                ws[dy, gi * 16:gi * 16 + 16, :] = wmat[:, :, dy, dx].T
        return np.ascontiguousarray(ws.transpose(1, 0, 2)).astype(F16)

    w1s = wstack(np.asarray(w1, np.float32))
    w2s = wstack(np.asarray(w2, np.float32), order=(1, 0, 2))
    b1t = np.asarray(b1, np.float32).reshape(16, 1).copy()
    b2t = np.asarray(b2, np.float32).reshape(16, 1).copy()
    ident = np.eye(16, dtype=F16)

    # band data bandG[p, blk, r, jh, kh, jl] (j-pair-interleaved so the
    # matmul rhs reads 4B-aligned element pairs at full fetch rate):
    #   j = 32*(p//32) - 2 + 2*jh + jl; w = j - 8; kw = p - j + 17
    #   yr = h0-2+4blk+r; xr = yr+kh-9; value = ker[b, kh*19+kw, yr, w]
    p_i = np.arange(128)[:, None, None, None, None, None]
    blk_i = np.arange(NBLK)[None, :, None, None, None, None]
    r_i = np.arange(4)[None, None, :, None, None, None]
    jh_i = np.arange(JH)[None, None, None, :, None, None]
    kh_i = np.arange(KK)[None, None, None, None, :, None]
    jl_i = np.arange(2)[None, None, None, None, None, :]
    j_i = (p_i // G) * G - 2 + 2 * jh_i + jl_i
    kw_i = p_i - j_i + 17
    w_i = j_i - 8
    yg_i = 4 * blk_i + r_i
    base_valid = (kw_i >= 0) & (kw_i < KK) & (w_i >= 0) & (w_i < W)
    kw_c = np.clip(kw_i, 0, KK - 1)
    w_c = np.clip(w_i, 0, W - 1)
    pk_i = kh_i * KK + kw_c

    in_maps = []
    for cid in range(NCORES):
        b = cid // 4
        h0 = 32 * (cid % 4)

        # dx-tripled input slab [48, NIN, 128]
        inp3 = np.zeros((48, NIN, W), np.float32)
        rows = inp_pad[b, :, h0:h0 + NIN, :]  # global rows [h0-12, h0+46)
        for dx in range(3):
            inp3[dx * 16:dx * 16 + 16, :, :] = rows[:, :, dx:dx + W]
        inp3 = inp3.astype(F16)

        yr_i = h0 - 2 + yg_i
        xr_i = yr_i + kh_i - 9
        valid = base_valid & (yr_i >= 0) & (yr_i < H) & (xr_i >= 0) & (xr_i < H)
        yr_c = np.clip(yr_i, 0, H - 1)
        bandG = (ker[b][pk_i, yr_c, w_c] * valid).astype(F16)

        in_maps.append({
            "inp3": np.ascontiguousarray(inp3.reshape(48, NIN * W)),
            "bandG": np.ascontiguousarray(
                bandG.reshape(128, NBLK * 4 * JH * KK * 2)),
            "w1s": np.ascontiguousarray(w1s.reshape(48, 3 * 16)),
            "w2s": np.ascontiguousarray(w2s.reshape(48, 3 * 16)),
            "b1t": b1t,
            "b2t": b2t,
            "ident": ident,
        })
    return in_maps


def _build_program():
    nc = bacc.Bacc("TRN2", target_bir_lowering=False, debug=False,
                   num_devices=NCORES)
    dt = mybir.dt

    inp3_d = nc.dram_tensor("inp3", [48, NIN * W], dt.float16,
                            kind="ExternalInput").ap()
    bandG_d = nc.dram_tensor("bandG", [128, NBLK * 4 * JH * KK * 2],
                             dt.float16, kind="ExternalInput").ap()
    w1s_d = nc.dram_tensor("w1s", [48, 3 * 16], dt.float16,
                           kind="ExternalInput").ap()
    w2s_d = nc.dram_tensor("w2s", [48, 3 * 16], dt.float16,
                           kind="ExternalInput").ap()
    b1t_d = nc.dram_tensor("b1t", [16, 1], dt.float32, kind="ExternalInput").ap()
    b2t_d = nc.dram_tensor("b2t", [16, 1], dt.float32, kind="ExternalInput").ap()
    ident_d = nc.dram_tensor("ident", [16, 16], dt.float16,
                             kind="ExternalInput").ap()
    out_d = nc.dram_tensor("out", [16, HS * W], dt.float16,
                           kind="ExternalOutput").ap()

    with tile.TileContext(nc) as tc:
        with ExitStack() as ctx:
            _body(ctx, tc, inp3_d, bandG_d, w1s_d, w2s_d, b1t_d, b2t_d,
                  ident_d, out_d)
    nc.compile()
    return nc


def _body(ctx, tc, inp3_d, bandG_d, w1s_d, w2s_d, b1t_d, b2t_d, ident_d,
          out_d):
    nc = tc.nc
    dt = mybir.dt
    AFT = mybir.ActivationFunctionType

    consts = ctx.enter_context(tc.tile_pool(name="consts", bufs=1))
    bigs = ctx.enter_context(tc.tile_pool(name="bigs", bufs=1))
    ps_c1 = ctx.enter_context(tc.tile_pool(name="ps_c1", bufs=3, space="PSUM"))
    ps_tp = ctx.enter_context(tc.tile_pool(name="ps_tp", bufs=1, space="PSUM"))
    ps_pp = ctx.enter_context(tc.tile_pool(name="ps_pp", bufs=2, space="PSUM"))
    ps_c2 = ctx.enter_context(tc.tile_pool(name="ps_c2", bufs=2, space="PSUM"))

    # ---- persistent SBUF tiles -------------------------------------------
    w1s_t = consts.tile([48, 3 * 16], dt.float16, tag="w1s")
    w2s_t = consts.tile([48, 3 * 16], dt.float16, tag="w2s")
    b1_t = consts.tile([16, 1], dt.float32, tag="b1")
    b2_t = consts.tile([16, 1], dt.float32, tag="b2")
    id_t = consts.tile([16, 16], dt.float16, tag="ident")
    inp3_t = bigs.tile([48, NIN * W], dt.float16, tag="inp3")
    x_t = bigs.tile([16, NX * W], dt.float16, tag="x")
    xT_t = bigs.tile([128, NX * 16], dt.float16, tag="xT")
    y3_t = bigs.tile([48, NY * W], dt.float16, tag="y3")
    out_t = bigs.tile([16, HS * W], dt.float16, tag="out")
    band = [bigs.tile([128, 4 * ROW], dt.float16, tag=f"band{i}",
                      name=f"band{i}") for i in range(NBUF)]
    band_v = [t[:].rearrange("p (r jh kh jl) -> p r jh kh jl",
                             r=4, jh=AW // 2, kh=KK, jl=2) for t in band]


    # ---- input loads (sync queue; inp3 split so conv1 rounds never wait) -
    C1, C2 = 20 * W, 36 * W
    nc.sync.dma_start(inp3_t[:, 0:C1], inp3_d[:, 0:C1])
    nc.sync.dma_start(w1s_t[:], w1s_d)
    nc.sync.dma_start(b1_t[:], b1t_d)
    nc.sync.dma_start(inp3_t[:, C1:C2], inp3_d[:, C1:C2])
    nc.sync.dma_start(inp3_t[:, C2:], inp3_d[:, C2:])
    nc.sync.dma_start(id_t[:], ident_d)

    # ---- one-time zeroing (wide bitcast views cut memset element count;
    # buffer 0 on vector, the rest on the otherwise-idle gpsimd queue so
    # the xT copies aren't stuck behind memsets in the vector FIFO) --------
    # only the matmul-read region (j-pairs [4, 68)) ever needs zeros
    for i in range(NBUF):
        eng = nc.vector if i % 2 == 0 else nc.gpsimd
        zv = band_v[i][:, :, 4:4 + W // 2, :, :]
        eng.memset(zv.bitcast(dt.float32), 0.0)
    y3_v = y3_t[:].rearrange("p (r w) -> p r w", r=NY)
    nc.vector.memset(y3_v[0:48, :, 0:1], 0.0)
    nc.vector.memset(y3_v[0:48, :, W - 1:W], 0.0)

    bandG_v = bandG_d.rearrange("p (blk r jh kh jl) -> p blk r jh kh jl",
                                blk=NBLK, r=4, jh=JH, kh=KK, jl=2)

    def band_chunk_dma(blk):
        # group g window: j in [32g-2, 32g+50) -> j_hi in [16g-1, 16g+25);
        # g=0 clips the two j<0 columns. y rows 0 (chunk 0) and 35 (chunk 8)
        # are never read by conv2, so skip their transfers (the pp matmuls
        # for them then see memset zeros / stale data, discarded anyway).
        dstt = band_v[blk % NBUF]
        r0, r1 = (1, 4) if blk == 0 else (0, 3) if blk == NBLK - 1 else (0, 4)
        # late chunks: split issue across both HWDGE rings (scalar's conv1
        # activation chain is long done by the time these are issued)
        engs = [nc.sync, nc.sync, nc.scalar, nc.scalar] if blk >= 5 \
            else [nc.sync] * NG
        for g in range(NG):
            jh0 = max(0, 16 * g - 1)
            s0 = 1 if g == 0 else 0
            engs[g].dma_start(
                dstt[G * g: G * g + G, r0:r1, jh0: 16 * g + 25, :, :],
                bandG_v[G * g: G * g + G, blk, r0:r1, s0:JH, :, :])

    for pre in range(NBUF):
        band_chunk_dma(pre)

    # ---- conv1 + GELU -> x, 4-way column-tiled ---------------------------
    inp3_v = inp3_t[:].rearrange("p (r w) -> p r w", r=NIN)
    x_v = x_t[:].rearrange("p (r w) -> p r w", r=NX)

    def transpose_batch(tb, nrows=8):
        # x rows [8tb, 8tb+nrows) -> xT cols, via PE transposes + DVE copy
        pt = ps_tp.tile([128, 128], dt.float16, tag="tp")
        for rr in range(nrows):
            nc.tensor.transpose(pt[:, 16 * rr: 16 * rr + 16],
                                x_v[:, 8 * tb + rr, :], id_t[:])
        nc.vector.tensor_copy(
            xT_t[:, 128 * tb: 128 * tb + 16 * nrows], pt[:, 0:16 * nrows])

    for sr in range(4):
        nsub = 4 if sr < 3 else 2
        psum = ps_c1.tile([128, 512], dt.float32, tag="c1")
        for sub in range(nsub):
            b4 = 4 * sr + sub
            for dy in range(3):
                nc.tensor.matmul(
                    psum[32 * sub: 32 * sub + 16, :],
                    w1s_t[:, dy * 16:(dy + 1) * 16],
                    inp3_v[:, 4 * b4 + dy: 4 * b4 + dy + 4, :],
                    start=(dy == 0), stop=(dy == 2),
                    tile_position=(0, 32 * sub))
        for sub in range(nsub):
            b4 = 4 * sr + sub
            nc.scalar.activation(x_t[:, 512 * b4: 512 * (b4 + 1)],
                                 psum[32 * sub: 32 * sub + 16, :],
                                 AFT.Gelu, bias=b1_t[:])
        if sr == 0:
            transpose_batch(0)     # x rows 0-7 (ACT blocks 0-1 done)
        if sr == 1:
            transpose_batch(1)     # rows 8-23 (ACT blocks 2-5)
            transpose_batch(2)
        if sr == 2:
            transpose_batch(3)     # rows 24-31 (ACT blocks 6-7)

    # ---- per-pixel conv + interleaved conv2 ------------------------------
    def conv2_round(blocks):
        psum = ps_c2.tile([128, 512], dt.float32, tag="c2")
        for sub, b in enumerate(blocks):
            for dy in range(3):
                nc.tensor.matmul(
                    psum[32 * sub: 32 * sub + 16, :],
                    w2s_t[:, dy * 16:(dy + 1) * 16],
                    y3_v[:, 4 * b + 1 + dy: 4 * b + 5 + dy, :],
                    start=(dy == 0), stop=(dy == 2),
                    tile_position=(0, 32 * sub))
        for sub, b in enumerate(blocks):
            nc.scalar.activation(out_t[:, 512 * b: 512 * (b + 1)],
                                 psum[32 * sub: 32 * sub + 16, :],
                                 AFT.Sigmoid, bias=b2_t[:])

    def shift_wave(r0, r1):
        # dx-shifted y copies for conv2 (edge cols stay zero from memset)
        nc.sync.dma_start(y3_v[16:32, r0:r1, 1:W], y3_v[0:16, r0:r1, 0:W - 1])
        nc.sync.dma_start(y3_v[32:48, r0:r1, 0:W - 1], y3_v[0:16, r0:r1, 1:W])

    for blk in range(NBLK):
        bt = band_v[blk % NBUF]
        pp = ps_pp.tile([128, 128], dt.float32, tag="pp")
        for t in range(22):
            xg = 4 * blk + t
            lhs = xT_t[:, 16 * xg: 16 * xg + 16]
            for g in range(max(0, t - 18), min(3, t) + 1):
                kh = t - g
                nc.tensor.matmul(
                    pp[32 * g: 32 * g + 16, :],
                    lhs,
                    bt[:, g, 4:4 + W // 2, kh, :],
                    start=(kh == 0), stop=(kh == KK - 1),
                    tile_position=(0, 32 * g))
        # evict 4 y rows (f32 psum -> fp16 y3)
        for g in range(4):
            yg = 4 * blk + g
            nc.vector.tensor_copy(y3_v[0:16, yg, :], pp[32 * g: 32 * g + 16, :])
        if blk + NBUF < NBLK:
            band_chunk_dma(blk + NBUF)
        # JIT transpose batches: blk3 needs x rows <=33 (tb4), blk5 <=41
        # (tb5), blk7 <=49 + blk8 <=53 (tb6)
        if blk == 0:
            transpose_batch(4)
        if blk == 2:
            transpose_batch(5)
        if blk == 4:
            transpose_batch(6, nrows=6)  # x rows 54,55 are never read
        if blk == 0:
            # conv2 weights aren't needed until ~mid-kernel; loading them
            # here (scalar queue, post-ACT-chain) keeps the early sync queue
            # free for the pp-gating band chunk 0
            nc.scalar.dma_start(w2s_t[:], w2s_d)
            nc.scalar.dma_start(b2_t[:], b2t_d)
        if blk == 6:
            shift_wave(0, 28)
            conv2_round([0, 1, 2, 3])
        if blk == 7:
            conv2_round([4, 5])
            nc.scalar.dma_start(out_d[:, 0:512 * 4], out_t[:, 0:512 * 4])

    shift_wave(28, NY)
    conv2_round([6, 7])
    nc.scalar.dma_start(out_d[:, 512 * 4:], out_t[:, 512 * 4:])


_NC_CACHE = None
LAST = {}


def _get_nc():
    global _NC_CACHE
    if _NC_CACHE is None:
        _NC_CACHE = _build_program()
    return _NC_CACHE


def kernel(input, kernel, w1, b1, w2, b2, _trace=False, _tmpdir=None):
    in_maps = _host_prepare(input, kernel, w1, b1, w2, b2)
    nc = _get_nc()
    res = run_bass_kernel_spmd(nc, in_maps, core_ids=list(range(NCORES)),
                               trace=_trace, tmpdir=_tmpdir)
    out = np.zeros((B, C, H, W), np.float32)
    for cid in range(NCORES):
        b = cid // 4
        h0 = 32 * (cid % 4)
        out[b, :, h0:h0 + HS, :] = (
            res.results[cid]["out"].astype(np.float32).reshape(16, HS, W))
    LAST["exec_ns"] = res.exec_time_ns
    LAST["trace"] = res.instructions_and_trace
    return out
